# revision 6
# baseline (speedup 1.0000x reference)
"""ComplEx decoder kernel for Trainium2 (8 NeuronCores, Bass/Tile).

scores[b,s,r,o] = Re( sum_c conj(x[b,s,c]) * R[r,o] * x[b,o,c] )
               = Gr[b,s,o]*Rr[r,o] - Gi[b,s,o]*Ri[r,o]
with Gr/Gi the complex Gram over the channel dim.

Strategy (per core, s-axis sharded 8 ways, 125 rows/core):
  1. Load host-pre-transposed xT[b] = [C=128, N=1000] (and the core's local
     s-slab xTl = [C, 125], plus -imag variant) straight into SBUF; one packed
     DMA so downstream instructions need a single DMA-sem wait.
  2. Gram matmuls on the PE: GrT/GiT [o-tile=125, s=125] per (b, o-tile).
  3. Apply R on the PE as diagonal-matrix matmuls:
       out[s, o-tile] = GrT_ot.T @ diag(Rr[r, o-tile]) + GiT_ot.T @ diag(-Ri[r, o-tile])
     Diagonal rhs blocks are built with one tensor_scalar per (r, o-tile):
       D = identity * R_col   (per-partition scalar multiply on the DVE).
     Blocks are padded to 128 columns (3 zero cols) so the DVE runs in 2x mode
     and the matmul moving operand is exactly N=512 (fp32 max) for r-chunks of 4.
  4. PSUM -> SBUF copy (ACT) with an AP permute to [s, r, o] layout, then one
     2 MB HWDGE DMA per (b, r-chunk) with 4 KB descriptors.

Each core receives the full xT plus its own 125-row s-slab; outputs are
concatenated on the host along s.
"""

import numpy as np

import concourse.bass as bass
import concourse.bacc as bacc
import concourse.mybir as mybir
from concourse.bass import ds
from concourse.bass_utils import run_bass_kernel_spmd
from concourse.tile import TileContext

f32 = mybir.dt.float32

B, N, C, R = 2, 1000, 128, 50
NCORES = 8
SLOC = N // NCORES  # 125 s-rows per core
NOT = 8             # number of o tiles
OTW = N // NOT      # o tile width = 125 (= matmul K)
DBW = 128           # D block width (3 zero pad cols): DVE 2x mode, matmul N=512
XB = 2 * N + 3 * SLOC  # packed x columns per batch
R_CHUNKS = [(0, 4), (4, 4), (8, 4), (12, 4), (16, 4), (20, 4), (24, 4),
            (28, 4), (32, 4), (36, 4), (40, 4), (44, 4), (48, 2)]


def build_program() -> bass.Bass:
    nc = bacc.Bacc()

    # Packed inputs (single DMA each so consumers wait on one DMA sem only):
    # xin[c, b*XB + 0:N]          = xT real   (= x_real[b, :, c])
    # xin[c, b*XB + N:2N]         = xT imag
    # xin[c, b*XB + 2N + ...]     = local xT real | local imag | -local imag
    # cst[p, 0:DBW]               = 125x128 identity (3 zero cols)
    # cst[p, DBW + ot*2R + r]     = R_real[r, ot*OTW+p]
    # cst[p, DBW + ot*2R + R + r] = -R_imag[r, ot*OTW+p]
    xin_d = nc.dram_tensor("xin", [C, B * XB], f32, kind="ExternalInput")
    cst_d = nc.dram_tensor("cst", [OTW, DBW + NOT * 2 * R], f32,
                           kind="ExternalInput")
    out = nc.dram_tensor("out", [B, SLOC, R, N], f32, kind="ExternalOutput")

    with TileContext(nc) as tc:
        with (
            tc.tile_pool(name="const", bufs=1) as constp,
            tc.tile_pool(name="xt", bufs=1) as xtp,
            tc.tile_pool(name="gt", bufs=1) as gtp,
        ):
            cst = constp.tile([OTW, DBW + NOT * 2 * R], f32, tag="cst")
            nc.sync.dma_start(out=cst[:, :], in_=cst_d[:, :])
            identd = cst[:, ds(0, DBW)]

            def rcol(ot, col):
                return cst[:, ds(DBW + ot * 2 * R + col, 1)]

            xin = xtp.tile([C, B * XB], f32, tag="xin")
            nc.sync.dma_start(out=xin[:, :], in_=xin_d[:, :])
            xT = [[xin[:, ds(b * XB + m * N, N)] for m in range(2)]
                  for b in range(B)]
            xTl = [[xin[:, ds(b * XB + 2 * N + m * SLOC, SLOC)] for m in range(2)]
                   for b in range(B)]
            xTl_in = [xin[:, ds(b * XB + 2 * N + 2 * SLOC, SLOC)] for b in range(B)]

            # ---- Gram matmuls: GrT/GiT [OTW, NOT, SLOC] per b ----
            GrT = [gtp.tile([OTW, NOT, SLOC], f32, tag=f"grt{b}", name=f"grt{b}")
                   for b in range(B)]
            GiT = [gtp.tile([OTW, NOT, SLOC], f32, tag=f"git{b}", name=f"git{b}")
                   for b in range(B)]
            with tc.tile_pool(name="psg", bufs=4, space="PSUM") as psgp:
                for b in range(B):
                    for ot in range(NOT):
                        lr = xT[b][0][:, ds(ot * OTW, OTW)]
                        li = xT[b][1][:, ds(ot * OTW, OTW)]
                        psg = psgp.tile([OTW, SLOC], f32, tag="psg")
                        nc.tensor.matmul(psg[:, :], lr, xTl[b][0],
                                         start=True, stop=False)
                        nc.tensor.matmul(psg[:, :], li, xTl[b][1],
                                         start=False, stop=True)
                        nc.scalar.copy(GrT[b][:, ot, :], psg[:, :])
                        psg2 = psgp.tile([OTW, SLOC], f32, tag="psg")
                        nc.tensor.matmul(psg2[:, :], li, xTl[b][0],
                                         start=True, stop=False)
                        nc.tensor.matmul(psg2[:, :], lr, xTl_in[b],
                                         start=False, stop=True)
                        nc.scalar.copy(GiT[b][:, ot, :], psg2[:, :])

            # ---- main loop: apply R via diagonal matmuls, stream out ----
            with (
                tc.tile_pool(name="dpool", bufs=8) as dp,
                tc.tile_pool(name="pso", bufs=2, space="PSUM") as psop,
                tc.tile_pool(name="osb", bufs=4) as osp,
            ):
                for r0, rc in R_CHUNKS:
                    osb = [osp.tile([SLOC, rc, N], f32, tag="osb", name="osb")
                           for _ in range(B)]
                    for half in range(2):
                        # D tiles for the 4 o-tiles of this half (shared by b)
                        drs, dis = [], []
                        for otl in range(4):
                            ot = half * 4 + otl
                            drt = dp.tile([OTW, rc * DBW], f32, tag="dr")
                            dit = dp.tile([OTW, rc * DBW], f32, tag="di")
                            for j in range(rc):
                                nc.vector.tensor_scalar_mul(
                                    drt[:, ds(j * DBW, DBW)], identd,
                                    rcol(ot, r0 + j),
                                )
                                nc.vector.tensor_scalar_mul(
                                    dit[:, ds(j * DBW, DBW)], identd,
                                    rcol(ot, R + r0 + j),
                                )
                            drs.append(drt)
                            dis.append(dit)
                        for b in range(B):
                            ps = psop.tile([SLOC, 4, 512], f32, tag="ps")
                            for otl in range(4):
                                ot = half * 4 + otl
                                pslice = ps[:, otl, ds(0, rc * DBW)]
                                nc.tensor.matmul(pslice, GrT[b][:, ot, :],
                                                 drs[otl][:, :], start=True,
                                                 stop=False)
                                nc.tensor.matmul(pslice, GiT[b][:, ot, :],
                                                 dis[otl][:, :], start=False,
                                                 stop=True)
                            src = ps[:, :, ds(0, rc * DBW)].rearrange(
                                "p a (r j) -> p r a j", r=rc, j=DBW
                            )[:, :, :, ds(0, OTW)]
                            dst = osb[b][:, :, ds(half * 4 * OTW, 4 * OTW)].rearrange(
                                "p r (a j) -> p r a j", a=4, j=OTW
                            )
                            nc.scalar.copy(dst, src)
                    for b in range(B):
                        nc.sync.dma_start(
                            out=out[b, :, ds(r0, rc), :], in_=osb[b][:, :, :]
                        )
    nc.compile()
    return nc


_PROG: bass.Bass | None = None


def _get_prog() -> bass.Bass:
    global _PROG
    if _PROG is None:
        _PROG = build_program()
    return _PROG


def _make_in_maps(x_real, x_imag, R_real, R_imag):
    x_real = np.asarray(x_real, dtype=np.float32)
    x_imag = np.asarray(x_imag, dtype=np.float32)
    rr = np.asarray(R_real, dtype=np.float32)
    ri = np.asarray(R_imag, dtype=np.float32)

    xt_r = x_real.transpose(0, 2, 1)   # [B, C, N]
    xt_i = x_imag.transpose(0, 2, 1)

    # cst: identity block + R columns
    cstarr = np.zeros((OTW, DBW + NOT * 2 * R), dtype=np.float32)
    cstarr[:, :DBW] = np.eye(OTW, DBW, dtype=np.float32)
    # rtmp[o, col]: col<R -> R_real[col, o]; col>=R -> -R_imag[col-R, o]
    rtmp = np.concatenate([rr.T, -ri.T], axis=1)           # [N, 2R]
    cstarr[:, DBW:] = (
        rtmp.reshape(NOT, OTW, 2 * R).transpose(1, 0, 2).reshape(OTW, NOT * 2 * R)
    )

    in_maps = []
    for c in range(NCORES):
        sl = slice(c * SLOC, (c + 1) * SLOC)
        xin = np.empty((C, B * XB), dtype=np.float32)
        for b in range(B):
            xin[:, b * XB: b * XB + N] = xt_r[b]
            xin[:, b * XB + N: b * XB + 2 * N] = xt_i[b]
            xin[:, b * XB + 2 * N: b * XB + 2 * N + SLOC] = xt_r[b][:, sl]
            xin[:, b * XB + 2 * N + SLOC: b * XB + 2 * N + 2 * SLOC] = xt_i[b][:, sl]
            xin[:, b * XB + 2 * N + 2 * SLOC: b * XB + XB] = -xt_i[b][:, sl]
        in_maps.append({"xin": xin, "cst": cstarr})
    return in_maps


def run_kernel(x_real, x_imag, R_real, R_imag, trace=False):
    """Returns (full_output, BassKernelResults)."""
    nc = _get_prog()
    in_maps = _make_in_maps(x_real, x_imag, R_real, R_imag)
    res = run_bass_kernel_spmd(nc, in_maps, core_ids=list(range(NCORES)),
                               trace=trace)
    full = np.concatenate([r["out"] for r in res.results], axis=1)
    return full, res


def kernel(x_real, x_imag, R_real, R_imag) -> np.ndarray:
    full, _ = run_kernel(x_real, x_imag, R_real, R_imag, trace=False)
    return full


# revision 8
# speedup vs baseline: 1.0520x; 1.0520x over previous
"""ComplEx decoder kernel for Trainium2 (8 NeuronCores, Bass/Tile).

scores[b,s,r,o] = Re( sum_c conj(x[b,s,c]) * R[r,o] * x[b,o,c] )
               = Gr[b,s,o]*Rr[r,o] - Gi[b,s,o]*Ri[r,o]
with Gr/Gi the complex Gram over the channel dim.

Strategy (per core, s-axis sharded 8 ways, 125 rows/core):
  1. Load host-pre-transposed xT[b] = [C=128, N=1000] (+ the core's local
     s-slab xTl = [C, 125] and -imag variant) in one packed DMA.
  2. Gram matmuls on the PE into stacked tiles Gst[b][ot] = [128, 125]:
     rows 0:64 = GrT, rows 64:128 = GiT for a 64-wide o-tile (using PE
     column-tiling so Gi lands on partitions 64:127 directly).
  3. Apply R on the PE as ONE fused matmul per (b, o-tile, r-chunk):
       out[s, (r, o)] = Gst.T @ D,  D[k, (r,j)] = delta(k,j)*Rr[r,o(j)]
                                              + delta(k-64,j)*(-Ri[r,o(j)])
     i.e. D columns stack diag(Rr) over diag(-Ri) — K=128 fully used, so the
     fp32 4-cycle/row tax is paid once instead of twice.  D tiles are built
     with one DVE tensor_scalar per (r, o-tile): stacked-identity * R-column
     (per-partition scalar), FD=64 (even -> 2x mode).  r-chunks of 8 give
     matmul N=512 (fp32 max, one PSUM bank).
  4. PSUM -> SBUF copies (split DVE/ACT) with an AP permute to [s, r, o]
     layout, then one 1-4 MB HWDGE DMA per (b, r-chunk), 4 KB descriptors.

Each core receives the full xT plus its own 125-row s-slab; outputs are
concatenated on the host along s.
"""

import numpy as np

import concourse.bass as bass
import concourse.bacc as bacc
import concourse.mybir as mybir
from concourse.bass import ds
from concourse.bass_utils import run_bass_kernel_spmd
from concourse.tile import TileContext

f32 = mybir.dt.float32

B, N, C, R = 2, 1000, 128, 50
NP = 1024            # o padded to 1024 so 64-wide o-tiles divide evenly
NCORES = 8
SLOC = N // NCORES   # 125 s-rows per core
OW = 64              # o tile width (stacked Gr/Gi -> K=128)
NT = NP // OW // 2   # 8 pairs of o-tiles (pair covers 128 o values)
XB = 2 * NP + 3 * SLOC
R_CHUNKS = [(0, 8), (8, 8), (16, 8), (24, 8), (32, 8), (40, 8), (48, 2)]


def build_program() -> bass.Bass:
    nc = bacc.Bacc()

    # Packed inputs:
    # xin[c, b*XB + 0:NP]   = xT real (o zero-padded to 1024)   (= x_real[b, :, c])
    # xin[c, b*XB + NP:2NP] = xT imag
    # xin[c, b*XB + 2N+...] = local xT real | local imag | -local imag
    # cst[p, 0:OW]          = stacked identity: 1 at (j, j) and (64+j, j)
    # cst[p, OW + ot*R + r] = R_real[r, ot*OW+p] if p < 64 else
    #                         -R_imag[r, ot*OW+p-64]
    xin_d = nc.dram_tensor("xin", [C, B * XB], f32, kind="ExternalInput")
    cst_d = nc.dram_tensor("cst", [C, OW + 2 * NT * R], f32, kind="ExternalInput")
    out = nc.dram_tensor("out", [B, SLOC, R, NP], f32, kind="ExternalOutput")

    with TileContext(nc) as tc:
        with (
            tc.tile_pool(name="const", bufs=1) as constp,
            tc.tile_pool(name="gt", bufs=1) as gtp,
        ):
            cst = constp.tile([C, OW + 2 * NT * R], f32, tag="cst")
            nc.sync.dma_start(out=cst[:, :], in_=cst_d[:, :])
            ident2 = cst[:, ds(0, OW)]

            def rcol(ot, r):
                return cst[:, ds(OW + ot * R + r, 1)]

            xin = constp.tile([C, B * XB], f32, tag="xin")
            nc.sync.dma_start(out=xin[:, :], in_=xin_d[:, :])
            xT = [[xin[:, ds(b * XB + m * NP, NP)] for m in range(2)]
                  for b in range(B)]
            xTl = [[xin[:, ds(b * XB + 2 * NP + m * SLOC, SLOC)] for m in range(2)]
                   for b in range(B)]
            xTl_in = [xin[:, ds(b * XB + 2 * NP + 2 * SLOC, SLOC)] for b in range(B)]

            # ---- Gram phase: Gst[b][ot] rows 0:64 = GrT, 64:128 = GiT ----
            Gst = gtp.tile([C, B * 2 * NT, SLOC], f32, tag="gst")
            with tc.tile_pool(name="psg", bufs=4, space="PSUM") as psgp:
                for b in range(B):
                    for ot in range(2 * NT):
                        lr = xT[b][0][:, ds(ot * OW, OW)]
                        li = xT[b][1][:, ds(ot * OW, OW)]
                        g = psgp.tile([C, SLOC], f32, tag="psg")
                        nc.tensor.matmul(g[0:OW, :], lr, xTl[b][0],
                                         start=True, stop=False,
                                         tile_position=(0, 0))
                        nc.tensor.matmul(g[0:OW, :], li, xTl[b][1],
                                         start=False, stop=True,
                                         tile_position=(0, 0))
                        nc.tensor.matmul(g[OW:C, :], li, xTl[b][0],
                                         start=True, stop=False,
                                         tile_position=(0, OW))
                        nc.tensor.matmul(g[OW:C, :], lr, xTl_in[b],
                                         start=False, stop=True,
                                         tile_position=(0, OW))
                        nc.scalar.copy(Gst[:, b * 2 * NT + ot, :], g[:, :])

            # ---- main loop: fused diag matmuls, stream out ----
            with (
                tc.tile_pool(name="dpool", bufs=8) as dp,
                tc.tile_pool(name="pso", bufs=4, space="PSUM") as psop,
                tc.tile_pool(name="osb", bufs=3) as osp,
            ):
                ncopy = 0
                for r0, rc in R_CHUNKS:
                    nn = rc * OW
                    osb = [osp.tile([SLOC, rc, NP], f32, tag="osb", name="osb")
                           for _ in range(B)]
                    for t in range(NT):
                        dpair = []
                        for i in range(2):
                            ot = 2 * t + i
                            d2 = dp.tile([C, nn], f32, tag="d2")
                            for j in range(rc):
                                nc.vector.tensor_scalar_mul(
                                    d2[:, ds(j * OW, OW)], ident2, rcol(ot, r0 + j)
                                )
                            dpair.append(d2)
                        for b in range(B):
                            ps = psop.tile([SLOC, 2, 512], f32, tag="ps")
                            for i in range(2):
                                nc.tensor.matmul(
                                    ps[:, i, ds(0, nn)],
                                    Gst[:, b * 2 * NT + 2 * t + i, :],
                                    dpair[i][:, :], start=True, stop=True,
                                )
                            # permute copy: src (i, r, j) -> dst (r, i, j)
                            src = ps[:, :, ds(0, nn)].rearrange(
                                "p i (r j) -> p r i j", r=rc, j=OW
                            )
                            dst = osb[b][:, :, ds(t * 2 * OW, 2 * OW)].rearrange(
                                "p r (i j) -> p r i j", i=2, j=OW
                            )
                            eng = nc.vector if (ncopy % 3 == 2) else nc.scalar
                            if eng is nc.vector:
                                nc.vector.tensor_copy(dst, src)
                            else:
                                nc.scalar.copy(dst, src)
                            ncopy += 1
                    for b in range(B):
                        nc.sync.dma_start(
                            out=out[b, :, ds(r0, rc), :], in_=osb[b][:, :, :]
                        )
    nc.compile()
    return nc


_PROG: bass.Bass | None = None


def _get_prog() -> bass.Bass:
    global _PROG
    if _PROG is None:
        _PROG = build_program()
    return _PROG


def _make_in_maps(x_real, x_imag, R_real, R_imag):
    x_real = np.asarray(x_real, dtype=np.float32)
    x_imag = np.asarray(x_imag, dtype=np.float32)
    rr = np.asarray(R_real, dtype=np.float32)
    ri = np.asarray(R_imag, dtype=np.float32)

    xt_r = np.zeros((B, C, NP), dtype=np.float32)
    xt_i = np.zeros((B, C, NP), dtype=np.float32)
    xt_r[:, :, :N] = x_real.transpose(0, 2, 1)
    xt_i[:, :, :N] = x_imag.transpose(0, 2, 1)

    cstarr = np.zeros((C, OW + 2 * NT * R), dtype=np.float32)
    eye = np.eye(OW, dtype=np.float32)
    cstarr[:OW, :OW] = eye
    cstarr[OW:, :OW] = eye
    # columns: [r, o] stacked: top 64 rows R_real[r, ot*OW+p], bottom -R_imag
    rrp = np.zeros((R, NP), dtype=np.float32)
    rip = np.zeros((R, NP), dtype=np.float32)
    rrp[:, :N] = rr
    rip[:, :N] = ri
    rt = rrp.T.reshape(2 * NT, OW, R)    # [ot, p, r]
    it = (-rip).T.reshape(2 * NT, OW, R)
    for ot in range(2 * NT):
        cstarr[:OW, OW + ot * R: OW + (ot + 1) * R] = rt[ot]
        cstarr[OW:, OW + ot * R: OW + (ot + 1) * R] = it[ot]

    in_maps = []
    for c in range(NCORES):
        sl = slice(c * SLOC, (c + 1) * SLOC)
        xin = np.empty((C, B * XB), dtype=np.float32)
        for b in range(B):
            xin[:, b * XB: b * XB + NP] = xt_r[b]
            xin[:, b * XB + NP: b * XB + 2 * NP] = xt_i[b]
            xin[:, b * XB + 2 * NP: b * XB + 2 * NP + SLOC] = xt_r[b][:, sl]
            xin[:, b * XB + 2 * NP + SLOC: b * XB + 2 * NP + 2 * SLOC] = xt_i[b][:, sl]
            xin[:, b * XB + 2 * NP + 2 * SLOC: b * XB + XB] = -xt_i[b][:, sl]
        in_maps.append({"xin": xin, "cst": cstarr})
    return in_maps


def run_kernel(x_real, x_imag, R_real, R_imag, trace=False):
    """Returns (full_output, BassKernelResults)."""
    nc = _get_prog()
    in_maps = _make_in_maps(x_real, x_imag, R_real, R_imag)
    res = run_bass_kernel_spmd(nc, in_maps, core_ids=list(range(NCORES)),
                               trace=trace)
    full = np.empty((B, N, R, N), dtype=np.float32)
    for c in range(NCORES):
        full[:, c * SLOC:(c + 1) * SLOC] = res.results[c]["out"][:, :, :, :N]
    return full, res


def kernel(x_real, x_imag, R_real, R_imag) -> np.ndarray:
    full, _ = run_kernel(x_real, x_imag, R_real, R_imag, trace=False)
    return full
